# revision 5
# baseline (speedup 1.0000x reference)
"""Multi-head causal self-attention on 8 Trainium2 NeuronCores.

Sharding: core c handles batch b = c//2 and head-group hg = c%2
(8 of 16 heads, i.e. 512 of 1024 head dims). Each core computes its
QKV projections, causal attention for its heads, and a partial output
projection (its 512 columns of wo). Host sums the two partials per
batch and adds bo.

All matmuls run in float32r (TF32-like, ~1e-4 rel err, 4x faster than
fp32 on the PE). Softmax skips max-subtraction (scores ~ N(0,1), safe
in fp32 exp) so attention is: expT = exp(scale*K^T Q) in k-major
layout; AV matmul contracts over k at full K=128 rate with an extra
ones-column in lhsT producing the softmax denominator for free.
"""

import sys

sys.path.insert(0, "/opt/trn_rl_repo")

import numpy as np

import concourse.bacc as bacc
import concourse.mybir as mybir
import concourse.tile as tile
from concourse.bass import ds
from concourse.masks import make_identity

B, S, D, H = 4, 2048, 1024, 16
DEPTH = D // H  # 64
N_CORES = 8
HPC = H // 2  # heads per core = 8
HD = HPC * DEPTH  # head dims per core = 512
NEG = -1e9
SCALE = 1.0 / np.sqrt(DEPTH)  # 0.125

P = 128
KT = D // P  # 8 contraction tiles over D
OTS = HD // P  # 4 out-dim tiles per projection
QB = 512  # q block size
NQB = S // QB  # 4
TOKT = S // P  # 16 token tiles
HDT = HD // P  # 4 head-dim contraction tiles for wo
VW = DEPTH + 1  # 65: v columns per head incl. ones column

f32 = mybir.dt.float32
f32r = mybir.dt.float32r
AF = mybir.ActivationFunctionType

_NC_CACHE = {}
LAST_RESULT = None


def _build_nc():
    nc = bacc.Bacc("TRN2", debug=False, num_devices=N_CORES)

    x_d = nc.dram_tensor("x", [S, D], f32, kind="ExternalInput").ap()
    wqt_d = nc.dram_tensor("wqt", [D, HD], f32r, kind="ExternalInput").ap()
    wkt_d = nc.dram_tensor("wkt", [D, HD], f32r, kind="ExternalInput").ap()
    wvt_d = nc.dram_tensor("wvt", [D, HD], f32r, kind="ExternalInput").ap()
    wot_d = nc.dram_tensor("wot", [HD, D], f32r, kind="ExternalInput").ap()
    bq_d = nc.dram_tensor("bq", [P, OTS], f32, kind="ExternalInput").ap()
    bk_d = nc.dram_tensor("bk", [P, OTS], f32, kind="ExternalInput").ap()
    bv_d = nc.dram_tensor("bv", [P, OTS], f32, kind="ExternalInput").ap()
    out_d = nc.dram_tensor("out", [S, D], f32, kind="ExternalOutput").ap()

    with tile.TileContext(nc) as tc:
        _emit(tc, nc, x_d, wqt_d, wkt_d, wvt_d, wot_d, bq_d, bk_d, bv_d, out_d)
    nc.finalize()
    return nc


def _emit(tc, nc, x_d, wqt_d, wkt_d, wvt_d, wot_d, bq_d, bk_d, bv_d, out_d):
    import contextlib

    with contextlib.ExitStack() as root:
        const = root.enter_context(tc.tile_pool(name="const", bufs=1))

        ident = const.tile([P, P], f32)
        make_identity(nc, ident)

        # diagonal-block causal masks, one per 128-row offset m within a
        # 512-col q block: masks[m][r, c] = NEG where 128m + r > c else 0
        masks = const.tile([P, 4, QB], f32)
        nc.gpsimd.memset(masks, 0.0)
        for m in range(4):
            nc.gpsimd.affine_select(
                out=masks[:, m, :],
                in_=masks[:, m, :],
                compare_op=mybir.AluOpType.is_ge,
                fill=NEG,
                base=-P * m,
                pattern=[[1, QB]],
                channel_multiplier=-1,
            )

        bq_sb = const.tile([P, OTS], f32)
        nc.sync.dma_start(bq_sb, bq_d)
        bk_sb = const.tile([P, OTS], f32)
        nc.sync.dma_start(bk_sb, bk_d)
        bv_sb = const.tile([P, OTS], f32)
        nc.sync.dma_start(bv_sb, bv_d)

        # ones row for the K=1 denominator-broadcast matmul
        ones_f = const.tile([1, DEPTH], f32)
        nc.vector.memset(ones_f, 1.0)
        ones_r = const.tile([1, DEPTH], f32r)
        nc.vector.tensor_copy(ones_r, ones_f)

        # persistent activations (f32r)
        qT = const.tile([P, OTS, S], f32r)  # [dim-in-tile, ot, token]
        kT = const.tile([P, OTS, S], f32r)
        v_sb = const.tile([P, TOKT, HPC * VW], f32r)  # token-major v + ones cols

        # ones columns of v_sb: positions 65h+64 for each head h, all kt
        ones_v_f = const.tile([P, TOKT, HPC], f32)
        nc.vector.memset(ones_v_f, 1.0)
        v_ones_ap = v_sb.rearrange("p t (h w) -> p t h w", w=VW)[:, :, :, DEPTH]
        nc.vector.tensor_copy(v_ones_ap, ones_v_f)

        # ---------------- phase 1: x transpose + QKV projections ------------
        with (
            tc.tile_pool(name="ph1", bufs=2) as ph1,
            tc.tile_pool(name="stage1", bufs=2) as stage1,
            tc.tile_pool(name="ps_t", bufs=3, space="PSUM") as ps_t,
            tc.tile_pool(name="ps_mm", bufs=3, space="PSUM") as ps_mm,
        ):
            for ch in range(NQB):  # 512-token chunks
                xT_ch = ph1.tile([P, KT, QB], f32r, tag="xT")
                for i in range(QB // P):
                    xt = stage1.tile([P, D], f32, tag="xstage")
                    nc.sync.dma_start(xt, x_d[ds(ch * QB + i * P, P), :])
                    for kt in range(KT):
                        pst = ps_t.tile([P, P], f32, tag="tp")
                        nc.tensor.transpose(pst, xt[:, ds(kt * P, P)], ident)
                        nc.vector.tensor_copy(xT_ch[:, kt, ds(i * P, P)], pst)

                for wt_d, bias_sb, dst in (
                    (wqt_d, bq_sb, qT),
                    (wkt_d, bk_sb, kT),
                    (wvt_d, bv_sb, None),
                ):
                    wT = ph1.tile([P, KT, HD], f32r, tag="wT")
                    nc.sync.dma_start(
                        wT, wt_d.rearrange("(kt p) m -> p kt m", p=P)
                    )
                    for ot in range(OTS):
                        psm = ps_mm.tile([P, QB], f32, tag="mm")
                        for kt in range(KT):
                            nc.tensor.matmul(
                                psm,
                                wT[:, kt, ds(ot * P, P)],
                                xT_ch[:, kt, :],
                                start=(kt == 0),
                                stop=(kt == KT - 1),
                            )
                        if dst is not None:
                            nc.vector.tensor_scalar_add(
                                dst[:, ot, ds(ch * QB, QB)],
                                psm,
                                bias_sb[:, ot : ot + 1],
                            )
                        else:
                            # v: bias add then transpose to token-major v_sb
                            vst = stage1.tile([P, QB], f32, tag="vstage")
                            nc.vector.tensor_scalar_add(
                                vst, psm, bv_sb[:, ot : ot + 1]
                            )
                            for j in range(QB // P):
                                psv = ps_t.tile([P, P], f32, tag="tp")
                                nc.tensor.transpose(
                                    psv, vst[:, ds(j * P, P)], ident
                                )
                                kt_tok = ch * (QB // P) + j
                                dst_ap = v_sb[
                                    :, kt_tok, ds(2 * VW * ot, 2 * VW)
                                ].rearrange("p (a w) -> p a w", w=VW)[:, :, :DEPTH]
                                nc.vector.tensor_copy(
                                    dst_ap,
                                    psv.rearrange("p (a w) -> p a w", w=DEPTH),
                                )

        # ---------------- phase 2: attention ---------------------------------
        ph2 = root.enter_context(tc.tile_pool(name="ph2", bufs=1))
        aoT = ph2.tile([P, HDT, S], f32r)  # normalized attn output, hd-major

        with (
            tc.tile_pool(name="ph2t", bufs=3) as ph2t,
            tc.tile_pool(name="ph2s", bufs=2) as ph2s,
            tc.tile_pool(name="ps_s", bufs=3, space="PSUM") as ps_s,
            tc.tile_pool(name="ps_av", bufs=3, space="PSUM") as ps_av,
            tc.tile_pool(name="ps_bc", bufs=1, space="PSUM") as ps_bc,
        ):
            for pr in range(HPC // 2):  # head pairs
                for qb in range(NQB):
                    n_kt = 4 * (qb + 1)
                    av = [
                        ps_av.tile([VW, QB], f32, tag="av", name=f"av_{pr}_{qb}_{ss}")
                        for ss in range(2)
                    ]
                    pend = []  # deferred AV emissions for pipelining
                    for kt in range(n_kt):
                        cur = []
                        for ss in range(2):
                            po = DEPTH * ss
                            pss = ps_s.tile([P, QB], f32, tag="s")
                            nc.tensor.matmul(
                                pss,
                                kT[po : po + DEPTH, pr, ds(kt * P, P)],
                                qT[po : po + DEPTH, pr, ds(qb * QB, QB)],
                                start=True,
                                stop=True,
                            )
                            cur.append(pss)
                        m = kt - 4 * qb
                        for ss in range(2):
                            if m >= 0:
                                nc.vector.tensor_add(
                                    cur[ss], cur[ss], masks[:, m, :]
                                )
                            expt = ph2t.tile([P, QB], f32r, tag=f"e{ss}")
                            nc.scalar.activation(expt, cur[ss], AF.Exp, scale=SCALE)
                            cur[ss] = expt
                        # defer AV by one kt so PE doesn't stall on ACT
                        pend.append((kt, cur))
                        if len(pend) > 1:
                            _emit_av(nc, v_sb, av, pend.pop(0), pr, n_kt)
                    while pend:
                        _emit_av(nc, v_sb, av, pend.pop(0), pr, n_kt)

                    # normalize: aoT = U * (1/rowsum) broadcast over 64 dims
                    for ss in range(2):
                        h = 2 * pr + ss
                        po = DEPTH * ss
                        rec = ph2s.tile([1, QB], f32, tag="rec")
                        nc.vector.reciprocal(rec, av[ss][DEPTH : DEPTH + 1, :])
                        recr = ph2s.tile([1, QB], f32r, tag="recr")
                        nc.vector.tensor_copy(recr, rec)
                        psb = ps_bc.tile([DEPTH, QB], f32, tag="bc")
                        nc.tensor.matmul(psb, ones_r, recr, start=True, stop=True)
                        rb = ph2s.tile([DEPTH, QB], f32, tag="rb")
                        nc.vector.tensor_copy(rb, psb)
                        nc.vector.tensor_mul(
                            aoT[po : po + DEPTH, pr, ds(qb * QB, QB)],
                            av[ss][:DEPTH, :],
                            rb,
                        )

        # ---------------- phase 3: output projection -------------------------
        with (
            tc.tile_pool(name="ph3", bufs=1) as ph3,
            tc.tile_pool(name="ostage", bufs=3) as ostage,
            tc.tile_pool(name="ps_o", bufs=2, space="PSUM") as ps_o,
        ):
            woT = ph3.tile([P, HDT, D], f32r)
            nc.sync.dma_start(woT, wot_d.rearrange("(kd p) m -> p kd m", p=P))
            for tt in range(TOKT):
                for oc in range(D // QB):
                    pso = ps_o.tile([P, QB], f32, tag="o")
                    for kd in range(HDT):
                        nc.tensor.matmul(
                            pso,
                            aoT[:, kd, ds(tt * P, P)],
                            woT[:, kd, ds(oc * QB, QB)],
                            start=(kd == 0),
                            stop=(kd == HDT - 1),
                        )
                    osb = ostage.tile([P, QB], f32, tag="os")
                    nc.vector.tensor_copy(osb, pso)
                    nc.sync.dma_start(
                        out_d[ds(tt * P, P), ds(oc * QB, QB)], osb
                    )


def _emit_av(nc, v_sb, av, item, pr, n_kt):
    kt, expts = item
    for ss in range(2):
        h = 2 * pr + ss
        nc.tensor.matmul(
            av[ss],
            v_sb[:, kt, ds(VW * h, VW)],
            expts[ss],
            start=(kt == 0),
            stop=(kt == n_kt - 1),
        )


def _get_nc():
    if "nc" not in _NC_CACHE:
        _NC_CACHE["nc"] = _build_nc()
    return _NC_CACHE["nc"]


def _get_runner():
    """Build the sharded jitted executable once and cache it.

    Mirrors bass2jax.run_bass_via_pjrt's multi-core branch, but without
    donation (the kernel writes every element of its outputs) so the same
    callable can be invoked repeatedly for timing.
    """
    if "runner" in _NC_CACHE:
        return _NC_CACHE["runner"]

    import jax
    from jax.experimental.shard_map import shard_map
    from jax.sharding import Mesh, PartitionSpec

    from concourse import bass2jax, mybir as _mybir

    bass2jax.install_neuronx_cc_hook()
    nc = _get_nc()

    partition_name = nc.partition_id_tensor.name if nc.partition_id_tensor else None
    in_names, out_names, out_avals, zero_outs = [], [], [], []
    for alloc in nc.m.functions[0].allocations:
        if not isinstance(alloc, _mybir.MemoryLocationSet):
            continue
        name = alloc.memorylocations[0].name
        if alloc.kind == "ExternalInput":
            if name != partition_name:
                in_names.append(name)
        elif alloc.kind == "ExternalOutput":
            shape = tuple(alloc.tensor_shape)
            dtype = _mybir.dt.np(alloc.dtype)
            out_names.append(name)
            out_avals.append(jax.core.ShapedArray(shape, dtype))
            zero_outs.append(np.zeros(shape, dtype))
    n_params = len(in_names)
    all_in_names = list(in_names) + list(out_names)
    if partition_name is not None:
        all_in_names.append(partition_name)

    def _body(*args):
        operands = list(args)
        if partition_name is not None:
            operands.append(bass2jax.partition_id_tensor())
        outs = bass2jax._bass_exec_p.bind(
            *operands,
            out_avals=tuple(out_avals),
            in_names=tuple(all_in_names),
            out_names=tuple(out_names),
            lowering_input_output_aliases=(),
            sim_require_finite=True,
            sim_require_nnan=True,
            nc=nc,
        )
        return tuple(outs)

    devices = jax.devices()[:N_CORES]
    mesh = Mesh(np.asarray(devices), ("core",))
    in_specs = (PartitionSpec("core"),) * (n_params + len(out_names))
    out_specs = (PartitionSpec("core"),) * len(out_names)
    sharded = jax.jit(
        shard_map(
            _body, mesh=mesh, in_specs=in_specs, out_specs=out_specs, check_rep=False
        ),
        keep_unused=True,
    )

    def run(in_maps):
        concat_in = [
            np.concatenate([m[name] for m in in_maps], axis=0) for name in in_names
        ]
        concat_zeros = [
            np.zeros((N_CORES * z.shape[0], *z.shape[1:]), z.dtype) for z in zero_outs
        ]
        out_arrs = sharded(*concat_in, *concat_zeros)
        return [
            {
                name: np.asarray(out_arrs[i]).reshape(
                    N_CORES, *out_avals[i].shape
                )[c]
                for i, name in enumerate(out_names)
            }
            for c in range(N_CORES)
        ]

    runner = {"run": run, "sharded": sharded, "in_names": in_names,
              "out_names": out_names, "out_avals": out_avals,
              "zero_outs": zero_outs, "mesh": mesh}
    _NC_CACHE["runner"] = runner
    return runner


def kernel(x, mask, wq, bq, wk, bk, wv, bv, wo, bo):
    global LAST_RESULT
    x = np.asarray(x, np.float32)
    wq = np.asarray(wq, np.float32)
    wk = np.asarray(wk, np.float32)
    wv = np.asarray(wv, np.float32)
    wo = np.asarray(wo, np.float32)
    bq = np.asarray(bq, np.float32)
    bk = np.asarray(bk, np.float32)
    bv = np.asarray(bv, np.float32)
    bo = np.asarray(bo, np.float32)

    in_maps = _make_in_maps(x, wq, bq, wk, bk, wv, bv, wo)
    results = _get_runner()["run"](in_maps)
    LAST_RESULT = results

    out = np.empty((B, S, D), np.float32)
    for b in range(B):
        out[b] = results[2 * b]["out"] + results[2 * b + 1]["out"] + bo
    return out


def _make_in_maps(x, wq, bq, wk, bk, wv, bv, wo):
    in_maps = []
    for c in range(N_CORES):
        b, hg = c // 2, c % 2
        rows = slice(HD * hg, HD * hg + HD)
        in_maps.append(
            {
                "x": np.ascontiguousarray(x[b]),
                "wqt": np.ascontiguousarray(wq[rows, :].T),
                "wkt": np.ascontiguousarray(wk[rows, :].T),
                "wvt": np.ascontiguousarray(wv[rows, :].T),
                "wot": np.ascontiguousarray(wo[:, rows].T),
                "bq": np.ascontiguousarray(bq[rows].reshape(OTS, P).T),
                "bk": np.ascontiguousarray(bk[rows].reshape(OTS, P).T),
                "bv": np.ascontiguousarray(bv[rows].reshape(OTS, P).T),
            }
        )
    return in_maps


def bench(inputs, iters=20):
    """Time pure device execution with device-resident inputs.

    Returns (times_s, results_of_last_run). Wall-clocks each sharded call
    with block_until_ready; min over iters approximates HW exec time +
    fixed dispatch overhead.
    """
    import time as _time

    import jax
    from jax.sharding import NamedSharding, PartitionSpec

    x = np.asarray(inputs["x"], np.float32)
    wq = np.asarray(inputs["wq"], np.float32)
    wk = np.asarray(inputs["wk"], np.float32)
    wv = np.asarray(inputs["wv"], np.float32)
    wo = np.asarray(inputs["wo"], np.float32)
    bq = np.asarray(inputs["bq"], np.float32)
    bk = np.asarray(inputs["bk"], np.float32)
    bv = np.asarray(inputs["bv"], np.float32)
    in_maps = _make_in_maps(x, wq, bq, wk, bk, wv, bv, wo)

    r = _get_runner()
    sharded, in_names, zero_outs, mesh = (
        r["sharded"], r["in_names"], r["zero_outs"], r["mesh"],
    )
    sh = NamedSharding(mesh, PartitionSpec("core"))
    dev_args = [
        jax.device_put(
            np.concatenate([m[name] for m in in_maps], axis=0), sh
        )
        for name in in_names
    ] + [
        jax.device_put(
            np.zeros((N_CORES * z.shape[0], *z.shape[1:]), z.dtype), sh
        )
        for z in zero_outs
    ]
    # warmup (also triggers compile on first use)
    out = sharded(*dev_args)
    jax.block_until_ready(out)
    times = []
    for _ in range(iters):
        t0 = _time.perf_counter()
        out = sharded(*dev_args)
        jax.block_until_ready(out)
        times.append(_time.perf_counter() - t0)
    return times, out


# revision 9
# speedup vs baseline: 1.2516x; 1.2516x over previous
"""Multi-head causal self-attention on 8 Trainium2 NeuronCores.

Sharding: core c handles batch b = c//2 and head-group hg = c%2
(8 of 16 heads, i.e. 512 of 1024 head dims). Each core computes its
QKV projections, causal attention for its heads, and a partial output
projection (its 512 columns of wo). Host sums the two partials per
batch and adds bo.

All matmuls run in float32r (TF32-like, ~1e-4 rel err, 4x faster than
fp32 on the PE). Softmax skips max-subtraction (scores ~ N(0,1), safe
in fp32 exp) so attention is: expT = exp(scale*K^T Q) in k-major
layout; AV matmul contracts over k at full K=128 rate with an extra
ones-column in lhsT producing the softmax denominator for free.
"""

import sys

import numpy as np

try:
    import concourse.bacc as _probe  # noqa: F401
except ModuleNotFoundError:
    sys.path.insert(0, "/opt/trn_rl_repo")

import concourse.bacc as bacc
import concourse.mybir as mybir
import concourse.tile as tile
from concourse.bass import ds
from concourse.masks import make_identity

B, S, D, H = 4, 2048, 1024, 16
DEPTH = D // H  # 64
N_CORES = 8
HPC = H // 2  # heads per core = 8
HD = HPC * DEPTH  # head dims per core = 512
NEG = -1e9
SCALE = 1.0 / np.sqrt(DEPTH)  # 0.125

P = 128
KT = D // P  # 8 contraction tiles over D
OTS = HD // P  # 4 out-dim tiles per projection
QB = 512  # q block size
NQB = S // QB  # 4
TOKT = S // P  # 16 token tiles
HDT = HD // P  # 4 head-dim contraction tiles for wo
VW = DEPTH + 1  # 65: v columns per head incl. ones column

f32 = mybir.dt.float32
f32r = mybir.dt.float32r
AF = mybir.ActivationFunctionType

_NC_CACHE = {}
LAST_RESULT = None


def _build_nc():
    nc = bacc.Bacc("TRN2", debug=False, num_devices=N_CORES)

    x_d = nc.dram_tensor("x", [S, D], f32, kind="ExternalInput").ap()
    wqt_d = nc.dram_tensor("wqt", [D, HD], f32r, kind="ExternalInput").ap()
    wkt_d = nc.dram_tensor("wkt", [D, HD], f32r, kind="ExternalInput").ap()
    wvt_d = nc.dram_tensor("wvt", [D, HD], f32r, kind="ExternalInput").ap()
    wot_d = nc.dram_tensor("wot", [HD, D], f32r, kind="ExternalInput").ap()
    bq_d = nc.dram_tensor("bq", [P, OTS], f32, kind="ExternalInput").ap()
    bk_d = nc.dram_tensor("bk", [P, OTS], f32, kind="ExternalInput").ap()
    bv_d = nc.dram_tensor("bv", [P, OTS], f32, kind="ExternalInput").ap()
    out_d = nc.dram_tensor("out", [S, D], f32, kind="ExternalOutput").ap()

    with tile.TileContext(nc) as tc:
        _emit(tc, nc, x_d, wqt_d, wkt_d, wvt_d, wot_d, bq_d, bk_d, bv_d, out_d)
    nc.finalize()
    return nc


def _emit(tc, nc, x_d, wqt_d, wkt_d, wvt_d, wot_d, bq_d, bk_d, bv_d, out_d):
    import contextlib

    with contextlib.ExitStack() as root:
        const = root.enter_context(tc.tile_pool(name="const", bufs=1))

        ident = const.tile([P, P], f32)
        make_identity(nc, ident)

        bq_sb = const.tile([P, OTS], f32)
        nc.sync.dma_start(bq_sb, bq_d)
        bk_sb = const.tile([P, OTS], f32)
        nc.sync.dma_start(bk_sb, bk_d)
        bv_sb = const.tile([P, OTS], f32)
        nc.sync.dma_start(bv_sb, bv_d)

        # ones row for the K=1 denominator-broadcast matmul
        ones_f = const.tile([1, DEPTH], f32)
        nc.vector.memset(ones_f, 1.0)
        ones_r = const.tile([1, DEPTH], f32r)
        nc.vector.tensor_copy(ones_r, ones_f)

        # persistent activations (f32r)
        qT = const.tile([P, OTS, S], f32r)  # [dim-in-tile, ot, token]
        kT = const.tile([P, OTS, S], f32r)
        v_sb = const.tile([P, TOKT, HPC * VW], f32r)  # token-major v + ones cols

        # ones columns of v_sb: positions 65h+64 for each head h, all kt
        ones_v_f = const.tile([P, TOKT, HPC], f32)
        nc.vector.memset(ones_v_f, 1.0)
        v_ones_ap = v_sb.rearrange("p t (h w) -> p t h w", w=VW)[:, :, :, DEPTH]
        nc.vector.tensor_copy(v_ones_ap, ones_v_f)

        # ---------------- phase 1: x transpose + QKV projections ------------
        with (
            tc.tile_pool(name="ph1", bufs=2) as ph1,
            tc.tile_pool(name="stage1", bufs=2) as stage1,
            tc.tile_pool(name="ps_t", bufs=3, space="PSUM") as ps_t,
            tc.tile_pool(name="ps_mm", bufs=3, space="PSUM") as ps_mm,
        ):
            for ch in range(NQB):  # 512-token chunks
                xT_ch = ph1.tile([P, KT, QB], f32r, tag="xT")
                for i in range(QB // P):
                    xt = stage1.tile([P, D], f32, tag="xstage")
                    nc.sync.dma_start(xt, x_d[ds(ch * QB + i * P, P), :])
                    for kt in range(KT):
                        pst = ps_t.tile([P, P], f32, tag="tp")
                        nc.tensor.transpose(pst, xt[:, ds(kt * P, P)], ident)
                        nc.vector.tensor_copy(xT_ch[:, kt, ds(i * P, P)], pst)

                for wt_d, bias_sb, dst in (
                    (wqt_d, bq_sb, qT),
                    (wkt_d, bk_sb, kT),
                    (wvt_d, bv_sb, None),
                ):
                    wT = ph1.tile([P, KT, HD], f32r, tag="wT")
                    nc.sync.dma_start(
                        wT, wt_d.rearrange("(kt p) m -> p kt m", p=P)
                    )
                    for ot in range(OTS):
                        psm = ps_mm.tile([P, QB], f32, tag="mm")
                        for kt in range(KT):
                            nc.tensor.matmul(
                                psm,
                                wT[:, kt, ds(ot * P, P)],
                                xT_ch[:, kt, :],
                                start=(kt == 0),
                                stop=(kt == KT - 1),
                            )
                        if dst is not None:
                            nc.vector.tensor_scalar_add(
                                dst[:, ot, ds(ch * QB, QB)],
                                psm,
                                bias_sb[:, ot : ot + 1],
                            )
                        else:
                            # v: bias add then transpose to token-major v_sb
                            vst = stage1.tile([P, QB], f32, tag="vstage")
                            nc.vector.tensor_scalar_add(
                                vst, psm, bv_sb[:, ot : ot + 1]
                            )
                            for j in range(QB // P):
                                psv = ps_t.tile([P, P], f32, tag="tp")
                                nc.tensor.transpose(
                                    psv, vst[:, ds(j * P, P)], ident
                                )
                                kt_tok = ch * (QB // P) + j
                                dst_ap = v_sb[
                                    :, kt_tok, ds(2 * VW * ot, 2 * VW)
                                ].rearrange("p (a w) -> p a w", w=VW)[:, :, :DEPTH]
                                nc.vector.tensor_copy(
                                    dst_ap,
                                    psv.rearrange("p (a w) -> p a w", w=DEPTH),
                                )

        # ---------------- phase 2: attention ---------------------------------
        ph2 = root.enter_context(tc.tile_pool(name="ph2", bufs=1))
        aoT = ph2.tile([P, HDT, S], f32r)  # normalized attn output, hd-major

        with (
            tc.tile_pool(name="ph2t", bufs=3) as ph2t,
            tc.tile_pool(name="ph2s", bufs=2) as ph2s,
            tc.tile_pool(name="ps_s", bufs=3, space="PSUM") as ps_s,
            tc.tile_pool(name="ps_av", bufs=3, space="PSUM") as ps_av,
            tc.tile_pool(name="ps_bc", bufs=1, space="PSUM") as ps_bc,
        ):
            for pr in range(HPC // 2):  # head pairs
                for qb in range(NQB):
                    n_kt = 4 * (qb + 1)
                    av = [
                        ps_av.tile([VW, QB], f32, tag="av", name=f"av_{pr}_{qb}_{ss}")
                        for ss in range(2)
                    ]
                    pend = []  # deferred AV emissions for pipelining
                    for kt in range(n_kt):
                        cur = []
                        for ss in range(2):
                            po = DEPTH * ss
                            pss = ps_s.tile([P, QB], f32, tag="s")
                            nc.tensor.matmul(
                                pss,
                                kT[po : po + DEPTH, pr, ds(kt * P, P)],
                                qT[po : po + DEPTH, pr, ds(qb * QB, QB)],
                                start=True,
                                stop=True,
                            )
                            cur.append(pss)
                        m = kt - 4 * qb
                        for ss in range(2):
                            expt = ph2t.tile([P, QB], f32r, tag=f"e{ss}")
                            nc.scalar.activation(expt, cur[ss], AF.Exp, scale=SCALE)
                            if m >= 0:
                                # zero the causally-masked (future) entries:
                                # keep where c - r - 128m >= 0
                                nc.gpsimd.affine_select(
                                    out=expt,
                                    in_=expt,
                                    compare_op=mybir.AluOpType.is_ge,
                                    fill=0.0,
                                    base=-P * m,
                                    pattern=[[1, QB]],
                                    channel_multiplier=-1,
                                )
                            cur[ss] = expt
                        # defer AV by one kt so PE doesn't stall on ACT
                        pend.append((kt, cur))
                        if len(pend) > 1:
                            _emit_av(nc, v_sb, av, pend.pop(0), pr, n_kt)
                    while pend:
                        _emit_av(nc, v_sb, av, pend.pop(0), pr, n_kt)

                    # normalize: aoT = U * (1/rowsum) broadcast over 64 dims
                    for ss in range(2):
                        h = 2 * pr + ss
                        po = DEPTH * ss
                        rec = ph2s.tile([1, QB], f32, tag="rec")
                        nc.vector.reciprocal(rec, av[ss][DEPTH : DEPTH + 1, :])
                        recr = ph2s.tile([1, QB], f32r, tag="recr")
                        nc.vector.tensor_copy(recr, rec)
                        psb = ps_bc.tile([DEPTH, QB], f32, tag="bc")
                        nc.tensor.matmul(psb, ones_r, recr, start=True, stop=True)
                        rb = ph2s.tile([DEPTH, QB], f32, tag="rb")
                        nc.vector.tensor_copy(rb, psb)
                        nc.vector.tensor_mul(
                            aoT[po : po + DEPTH, pr, ds(qb * QB, QB)],
                            av[ss][:DEPTH, :],
                            rb,
                        )

        # ---------------- phase 3: output projection -------------------------
        with (
            tc.tile_pool(name="ph3", bufs=1) as ph3,
            tc.tile_pool(name="ostage", bufs=3) as ostage,
            tc.tile_pool(name="ps_o", bufs=2, space="PSUM") as ps_o,
        ):
            woT = ph3.tile([P, HDT, D], f32r)
            nc.sync.dma_start(woT, wot_d.rearrange("(kd p) m -> p kd m", p=P))
            for tt in range(TOKT):
                for oc in range(D // QB):
                    pso = ps_o.tile([P, QB], f32, tag="o")
                    for kd in range(HDT):
                        nc.tensor.matmul(
                            pso,
                            aoT[:, kd, ds(tt * P, P)],
                            woT[:, kd, ds(oc * QB, QB)],
                            start=(kd == 0),
                            stop=(kd == HDT - 1),
                        )
                    osb = ostage.tile([P, QB], f32, tag="os")
                    nc.scalar.copy(osb, pso)
                    nc.sync.dma_start(
                        out_d[ds(tt * P, P), ds(oc * QB, QB)], osb
                    )


def _emit_av(nc, v_sb, av, item, pr, n_kt):
    kt, expts = item
    for ss in range(2):
        h = 2 * pr + ss
        nc.tensor.matmul(
            av[ss],
            v_sb[:, kt, ds(VW * h, VW)],
            expts[ss],
            start=(kt == 0),
            stop=(kt == n_kt - 1),
        )


def _get_nc():
    if "nc" not in _NC_CACHE:
        _NC_CACHE["nc"] = _build_nc()
    return _NC_CACHE["nc"]


def _get_runner():
    """Build the sharded jitted executable once and cache it.

    Mirrors bass2jax.run_bass_via_pjrt's multi-core branch, but without
    donation (the kernel writes every element of its outputs) so the same
    callable can be invoked repeatedly for timing.
    """
    if "runner" in _NC_CACHE:
        return _NC_CACHE["runner"]

    import jax
    from jax.experimental.shard_map import shard_map
    from jax.sharding import Mesh, PartitionSpec

    from concourse import bass2jax, mybir as _mybir

    bass2jax.install_neuronx_cc_hook()
    nc = _get_nc()

    partition_name = nc.partition_id_tensor.name if nc.partition_id_tensor else None
    in_names, out_names, out_avals, zero_outs = [], [], [], []
    for alloc in nc.m.functions[0].allocations:
        if not isinstance(alloc, _mybir.MemoryLocationSet):
            continue
        name = alloc.memorylocations[0].name
        if alloc.kind == "ExternalInput":
            if name != partition_name:
                in_names.append(name)
        elif alloc.kind == "ExternalOutput":
            shape = tuple(alloc.tensor_shape)
            dtype = _mybir.dt.np(alloc.dtype)
            out_names.append(name)
            out_avals.append(jax.core.ShapedArray(shape, dtype))
            zero_outs.append(np.zeros(shape, dtype))
    n_params = len(in_names)
    all_in_names = list(in_names) + list(out_names)
    if partition_name is not None:
        all_in_names.append(partition_name)

    def _body(*args):
        operands = list(args)
        if partition_name is not None:
            operands.append(bass2jax.partition_id_tensor())
        outs = bass2jax._bass_exec_p.bind(
            *operands,
            out_avals=tuple(out_avals),
            in_names=tuple(all_in_names),
            out_names=tuple(out_names),
            lowering_input_output_aliases=(),
            sim_require_finite=True,
            sim_require_nnan=True,
            nc=nc,
        )
        return tuple(outs)

    devices = jax.devices()[:N_CORES]
    mesh = Mesh(np.asarray(devices), ("core",))
    in_specs = (PartitionSpec("core"),) * (n_params + len(out_names))
    out_specs = (PartitionSpec("core"),) * len(out_names)
    sharded = jax.jit(
        shard_map(
            _body, mesh=mesh, in_specs=in_specs, out_specs=out_specs, check_rep=False
        ),
        keep_unused=True,
    )

    def run(in_maps):
        concat_in = [
            np.concatenate([m[name] for m in in_maps], axis=0) for name in in_names
        ]
        concat_zeros = [
            np.zeros((N_CORES * z.shape[0], *z.shape[1:]), z.dtype) for z in zero_outs
        ]
        out_arrs = sharded(*concat_in, *concat_zeros)
        return [
            {
                name: np.asarray(out_arrs[i]).reshape(
                    N_CORES, *out_avals[i].shape
                )[c]
                for i, name in enumerate(out_names)
            }
            for c in range(N_CORES)
        ]

    runner = {"run": run, "sharded": sharded, "in_names": in_names,
              "out_names": out_names, "out_avals": out_avals,
              "zero_outs": zero_outs, "mesh": mesh}
    _NC_CACHE["runner"] = runner
    return runner


def kernel(x, mask, wq, bq, wk, bk, wv, bv, wo, bo):
    global LAST_RESULT
    x = np.asarray(x, np.float32)
    wq = np.asarray(wq, np.float32)
    wk = np.asarray(wk, np.float32)
    wv = np.asarray(wv, np.float32)
    wo = np.asarray(wo, np.float32)
    bq = np.asarray(bq, np.float32)
    bk = np.asarray(bk, np.float32)
    bv = np.asarray(bv, np.float32)
    bo = np.asarray(bo, np.float32)

    in_maps = _make_in_maps(x, wq, bq, wk, bk, wv, bv, wo)
    results = _get_runner()["run"](in_maps)
    LAST_RESULT = results

    out = np.empty((B, S, D), np.float32)
    for b in range(B):
        out[b] = results[2 * b]["out"] + results[2 * b + 1]["out"] + bo
    return out


def _make_in_maps(x, wq, bq, wk, bk, wv, bv, wo):
    in_maps = []
    for c in range(N_CORES):
        b, hg = c // 2, c % 2
        rows = slice(HD * hg, HD * hg + HD)
        in_maps.append(
            {
                "x": np.ascontiguousarray(x[b]),
                "wqt": np.ascontiguousarray(wq[rows, :].T),
                "wkt": np.ascontiguousarray(wk[rows, :].T),
                "wvt": np.ascontiguousarray(wv[rows, :].T),
                "wot": np.ascontiguousarray(wo[:, rows].T),
                "bq": np.ascontiguousarray(bq[rows].reshape(OTS, P).T),
                "bk": np.ascontiguousarray(bk[rows].reshape(OTS, P).T),
                "bv": np.ascontiguousarray(bv[rows].reshape(OTS, P).T),
            }
        )
    return in_maps


def bench(inputs, iters=20):
    """Time pure device execution with device-resident inputs.

    Returns (times_s, results_of_last_run). Wall-clocks each sharded call
    with block_until_ready; min over iters approximates HW exec time +
    fixed dispatch overhead.
    """
    import time as _time

    import jax
    from jax.sharding import NamedSharding, PartitionSpec

    x = np.asarray(inputs["x"], np.float32)
    wq = np.asarray(inputs["wq"], np.float32)
    wk = np.asarray(inputs["wk"], np.float32)
    wv = np.asarray(inputs["wv"], np.float32)
    wo = np.asarray(inputs["wo"], np.float32)
    bq = np.asarray(inputs["bq"], np.float32)
    bk = np.asarray(inputs["bk"], np.float32)
    bv = np.asarray(inputs["bv"], np.float32)
    in_maps = _make_in_maps(x, wq, bq, wk, bk, wv, bv, wo)

    r = _get_runner()
    sharded, in_names, zero_outs, mesh = (
        r["sharded"], r["in_names"], r["zero_outs"], r["mesh"],
    )
    sh = NamedSharding(mesh, PartitionSpec("core"))
    dev_args = [
        jax.device_put(
            np.concatenate([m[name] for m in in_maps], axis=0), sh
        )
        for name in in_names
    ] + [
        jax.device_put(
            np.zeros((N_CORES * z.shape[0], *z.shape[1:]), z.dtype), sh
        )
        for z in zero_outs
    ]
    # warmup (also triggers compile on first use)
    out = sharded(*dev_args)
    jax.block_until_ready(out)
    times = []
    for _ in range(iters):
        t0 = _time.perf_counter()
        out = sharded(*dev_args)
        jax.block_until_ready(out)
        times.append(_time.perf_counter() - t0)
    return times, out


# revision 12
# speedup vs baseline: 1.2523x; 1.0005x over previous
"""Multi-head causal self-attention on 8 Trainium2 NeuronCores.

Sharding: core c handles batch b = c//2 and head-group hg = c%2
(8 of 16 heads, i.e. 512 of 1024 head dims). Each core computes its
QKV projections, causal attention for its heads, and a partial output
projection (its 512 columns of wo). Host sums the two partials per
batch and adds bo.

All matmuls run in float32r (TF32-like, ~1e-4 rel err, 4x faster than
fp32 on the PE). Softmax skips max-subtraction (scores ~ N(0,1), safe
in fp32 exp) so attention is: expT = exp(scale*K^T Q) in k-major
layout; AV matmul contracts over k at full K=128 rate with an extra
ones-column in lhsT producing the softmax denominator for free.
"""

import sys

import numpy as np

try:
    import concourse.bacc as _probe  # noqa: F401
except ModuleNotFoundError:
    sys.path.insert(0, "/opt/trn_rl_repo")

import concourse.bacc as bacc
import concourse.mybir as mybir
import concourse.tile as tile
from concourse.bass import ds
from concourse.masks import make_identity

B, S, D, H = 4, 2048, 1024, 16
DEPTH = D // H  # 64
N_CORES = 8
HPC = H // 2  # heads per core = 8
HD = HPC * DEPTH  # head dims per core = 512
NEG = -1e9
SCALE = 1.0 / np.sqrt(DEPTH)  # 0.125

P = 128
KT = D // P  # 8 contraction tiles over D
OTS = HD // P  # 4 out-dim tiles per projection
QB = 512  # q block size
NQB = S // QB  # 4
TOKT = S // P  # 16 token tiles
HDT = HD // P  # 4 head-dim contraction tiles for wo
VW = DEPTH + 1  # 65: v columns per head incl. ones column

f32 = mybir.dt.float32
f32r = mybir.dt.float32r
bf16 = mybir.dt.bfloat16
AF = mybir.ActivationFunctionType

# compute dtype for matmul operands: "f32r" (TF32-like, ~2.6e-4 end-to-end
# error) or "bf16" (faster weight loads via FWL, ~few e-3 error)
import os as _os

CDT_NAME = _os.environ.get("KERNEL_DTYPE", "f32r")
CDT = {"f32r": f32r, "bf16": bf16}[CDT_NAME]

_NC_CACHE = {}
LAST_RESULT = None


def _build_nc():
    nc = bacc.Bacc("TRN2", debug=False, num_devices=N_CORES)

    x_d = nc.dram_tensor("x", [S, D], f32, kind="ExternalInput").ap()
    wqt_d = nc.dram_tensor("wqt", [D, HD], CDT, kind="ExternalInput").ap()
    wkt_d = nc.dram_tensor("wkt", [D, HD], CDT, kind="ExternalInput").ap()
    wvt_d = nc.dram_tensor("wvt", [D, HD], CDT, kind="ExternalInput").ap()
    wot_d = nc.dram_tensor("wot", [HD, D], CDT, kind="ExternalInput").ap()
    bq_d = nc.dram_tensor("bq", [P, OTS], f32, kind="ExternalInput").ap()
    bk_d = nc.dram_tensor("bk", [P, OTS], f32, kind="ExternalInput").ap()
    bv_d = nc.dram_tensor("bv", [P, OTS], f32, kind="ExternalInput").ap()
    out_d = nc.dram_tensor("out", [S, D], f32, kind="ExternalOutput").ap()

    with tile.TileContext(nc) as tc:
        _emit(tc, nc, x_d, wqt_d, wkt_d, wvt_d, wot_d, bq_d, bk_d, bv_d, out_d)
    nc.finalize()
    return nc


def _emit(tc, nc, x_d, wqt_d, wkt_d, wvt_d, wot_d, bq_d, bk_d, bv_d, out_d):
    import contextlib

    with contextlib.ExitStack() as root:
        const = root.enter_context(tc.tile_pool(name="const", bufs=1))

        ident = const.tile([P, P], f32)
        make_identity(nc, ident)

        bq_sb = const.tile([P, OTS], f32)
        nc.sync.dma_start(bq_sb, bq_d)
        bk_sb = const.tile([P, OTS], f32)
        nc.sync.dma_start(bk_sb, bk_d)
        bv_sb = const.tile([P, OTS], f32)
        nc.sync.dma_start(bv_sb, bv_d)

        # ones row for the K=1 denominator-broadcast matmul
        ones_f = const.tile([1, DEPTH], f32)
        nc.vector.memset(ones_f, 1.0)
        ones_r = const.tile([1, DEPTH], f32r)
        nc.vector.tensor_copy(ones_r, ones_f)

        # persistent activations (f32r)
        qT = const.tile([P, OTS, S], CDT)  # [dim-in-tile, ot, token]
        kT = const.tile([P, OTS, S], CDT)
        v_sb = const.tile([P, TOKT, HPC * VW], CDT)  # token-major v + ones cols

        # ones columns of v_sb: positions 65h+64 for each head h, all kt
        ones_v_f = const.tile([P, TOKT, HPC], f32)
        nc.vector.memset(ones_v_f, 1.0)
        v_ones_ap = v_sb.rearrange("p t (h w) -> p t h w", w=VW)[:, :, :, DEPTH]
        nc.vector.tensor_copy(v_ones_ap, ones_v_f)

        # ---------------- phase 1: x transpose + QKV projections ------------
        with (
            tc.tile_pool(name="ph1", bufs=2) as ph1,
            tc.tile_pool(name="stage1", bufs=2) as stage1,
            tc.tile_pool(name="ps_t", bufs=3, space="PSUM") as ps_t,
            tc.tile_pool(name="ps_mm", bufs=3, space="PSUM") as ps_mm,
        ):
            for ch in range(NQB):  # 512-token chunks
                xT_ch = ph1.tile([P, KT, QB], CDT, tag="xT")
                for i in range(QB // P):
                    xt = stage1.tile([P, D], f32, tag="xstage")
                    nc.sync.dma_start(xt, x_d[ds(ch * QB + i * P, P), :])
                    for kt in range(KT):
                        pst = ps_t.tile([P, P], f32, tag="tp")
                        nc.tensor.transpose(pst, xt[:, ds(kt * P, P)], ident)
                        nc.vector.tensor_copy(xT_ch[:, kt, ds(i * P, P)], pst)

                for wt_d, bias_sb, dst in (
                    (wqt_d, bq_sb, qT),
                    (wkt_d, bk_sb, kT),
                    (wvt_d, bv_sb, None),
                ):
                    wT = ph1.tile([P, KT, HD], CDT, tag="wT")
                    nc.sync.dma_start(
                        wT, wt_d.rearrange("(kt p) m -> p kt m", p=P)
                    )
                    for ot in range(OTS):
                        psm = ps_mm.tile([P, QB], f32, tag="mm")
                        for kt in range(KT):
                            nc.tensor.matmul(
                                psm,
                                wT[:, kt, ds(ot * P, P)],
                                xT_ch[:, kt, :],
                                start=(kt == 0),
                                stop=(kt == KT - 1),
                            )
                        if dst is not None:
                            nc.vector.tensor_scalar_add(
                                dst[:, ot, ds(ch * QB, QB)],
                                psm,
                                bias_sb[:, ot : ot + 1],
                            )
                        else:
                            # v: bias add then transpose to token-major v_sb
                            vst = stage1.tile([P, QB], f32, tag="vstage")
                            nc.vector.tensor_scalar_add(
                                vst, psm, bv_sb[:, ot : ot + 1]
                            )
                            for j in range(QB // P):
                                psv = ps_t.tile([P, P], f32, tag="tp")
                                nc.tensor.transpose(
                                    psv, vst[:, ds(j * P, P)], ident
                                )
                                kt_tok = ch * (QB // P) + j
                                dst_ap = v_sb[
                                    :, kt_tok, ds(2 * VW * ot, 2 * VW)
                                ].rearrange("p (a w) -> p a w", w=VW)[:, :, :DEPTH]
                                nc.vector.tensor_copy(
                                    dst_ap,
                                    psv.rearrange("p (a w) -> p a w", w=DEPTH),
                                )

        # ---------------- phase 2: attention ---------------------------------
        ph2 = root.enter_context(tc.tile_pool(name="ph2", bufs=1))
        aoT = ph2.tile([P, HDT, S], CDT)  # normalized attn output, hd-major

        with (
            tc.tile_pool(name="ph2t", bufs=3) as ph2t,
            tc.tile_pool(name="ph2s", bufs=2) as ph2s,
            tc.tile_pool(name="ps_s", bufs=3, space="PSUM") as ps_s,
            tc.tile_pool(name="ps_av", bufs=3, space="PSUM") as ps_av,
            tc.tile_pool(name="ps_bc", bufs=1, space="PSUM") as ps_bc,
        ):
            for pr in range(HPC // 2):  # head pairs
                for qb in range(NQB):
                    n_kt = 4 * (qb + 1)
                    av = [
                        ps_av.tile([VW, QB], f32, tag="av", name=f"av_{pr}_{qb}_{ss}")
                        for ss in range(2)
                    ]
                    pend = []  # deferred AV emissions for pipelining
                    for kt in range(n_kt):
                        cur = []
                        for ss in range(2):
                            po = DEPTH * ss
                            pss = ps_s.tile([P, QB], f32, tag="s")
                            nc.tensor.matmul(
                                pss,
                                kT[po : po + DEPTH, pr, ds(kt * P, P)],
                                qT[po : po + DEPTH, pr, ds(qb * QB, QB)],
                                start=True,
                                stop=True,
                            )
                            cur.append(pss)
                        m = kt - 4 * qb
                        for ss in range(2):
                            expt = ph2t.tile([P, QB], CDT, tag=f"e{ss}")
                            nc.scalar.activation(expt, cur[ss], AF.Exp, scale=SCALE)
                            if m >= 0:
                                # zero the causally-masked (future) entries:
                                # keep where c - r - 128m >= 0
                                nc.gpsimd.affine_select(
                                    out=expt,
                                    in_=expt,
                                    compare_op=mybir.AluOpType.is_ge,
                                    fill=0.0,
                                    base=-P * m,
                                    pattern=[[1, QB]],
                                    channel_multiplier=-1,
                                )
                            cur[ss] = expt
                        # defer AV by one kt so PE doesn't stall on ACT
                        pend.append((kt, cur))
                        if len(pend) > 1:
                            _emit_av(nc, v_sb, av, pend.pop(0), pr, n_kt)
                    while pend:
                        _emit_av(nc, v_sb, av, pend.pop(0), pr, n_kt)

                    # normalize: aoT = U * (1/rowsum) broadcast over 64 dims
                    for ss in range(2):
                        h = 2 * pr + ss
                        po = DEPTH * ss
                        rec = ph2s.tile([1, QB], f32, tag="rec")
                        nc.vector.reciprocal(rec, av[ss][DEPTH : DEPTH + 1, :])
                        recr = ph2s.tile([1, QB], f32r, tag="recr")
                        nc.vector.tensor_copy(recr, rec)
                        psb = ps_bc.tile([DEPTH, QB], f32, tag="bc")
                        nc.tensor.matmul(psb, ones_r, recr, start=True, stop=True)
                        rb = ph2s.tile([DEPTH, QB], f32, tag="rb")
                        nc.vector.tensor_copy(rb, psb)
                        nc.vector.tensor_mul(
                            aoT[po : po + DEPTH, pr, ds(qb * QB, QB)],
                            av[ss][:DEPTH, :],
                            rb,
                        )

        # ---------------- phase 3: output projection -------------------------
        with (
            tc.tile_pool(name="ph3", bufs=1) as ph3,
            tc.tile_pool(name="ostage", bufs=3) as ostage,
            tc.tile_pool(name="ps_o", bufs=2, space="PSUM") as ps_o,
        ):
            woT = ph3.tile([P, HDT, D], CDT)
            nc.sync.dma_start(woT, wot_d.rearrange("(kd p) m -> p kd m", p=P))
            for tt in range(TOKT):
                for oc in range(D // QB):
                    pso = ps_o.tile([P, QB], f32, tag="o")
                    for kd in range(HDT):
                        nc.tensor.matmul(
                            pso,
                            aoT[:, kd, ds(tt * P, P)],
                            woT[:, kd, ds(oc * QB, QB)],
                            start=(kd == 0),
                            stop=(kd == HDT - 1),
                        )
                    osb = ostage.tile([P, QB], f32, tag="os")
                    nc.scalar.copy(osb, pso)
                    nc.sync.dma_start(
                        out_d[ds(tt * P, P), ds(oc * QB, QB)], osb
                    )


def _emit_av(nc, v_sb, av, item, pr, n_kt):
    kt, expts = item
    for ss in range(2):
        h = 2 * pr + ss
        nc.tensor.matmul(
            av[ss],
            v_sb[:, kt, ds(VW * h, VW)],
            expts[ss],
            start=(kt == 0),
            stop=(kt == n_kt - 1),
        )


def _get_nc():
    if "nc" not in _NC_CACHE:
        _NC_CACHE["nc"] = _build_nc()
    return _NC_CACHE["nc"]


def _get_runner():
    """Build the sharded jitted executable once and cache it.

    Mirrors bass2jax.run_bass_via_pjrt's multi-core branch, but without
    donation (the kernel writes every element of its outputs) so the same
    callable can be invoked repeatedly for timing.
    """
    if "runner" in _NC_CACHE:
        return _NC_CACHE["runner"]

    import jax
    from jax.experimental.shard_map import shard_map
    from jax.sharding import Mesh, PartitionSpec

    from concourse import bass2jax, mybir as _mybir

    bass2jax.install_neuronx_cc_hook()
    nc = _get_nc()

    partition_name = nc.partition_id_tensor.name if nc.partition_id_tensor else None
    in_names, out_names, out_avals, zero_outs = [], [], [], []
    for alloc in nc.m.functions[0].allocations:
        if not isinstance(alloc, _mybir.MemoryLocationSet):
            continue
        name = alloc.memorylocations[0].name
        if alloc.kind == "ExternalInput":
            if name != partition_name:
                in_names.append(name)
        elif alloc.kind == "ExternalOutput":
            shape = tuple(alloc.tensor_shape)
            dtype = _mybir.dt.np(alloc.dtype)
            out_names.append(name)
            out_avals.append(jax.core.ShapedArray(shape, dtype))
            zero_outs.append(np.zeros(shape, dtype))
    n_params = len(in_names)
    all_in_names = list(in_names) + list(out_names)
    if partition_name is not None:
        all_in_names.append(partition_name)

    def _body(*args):
        operands = list(args)
        if partition_name is not None:
            operands.append(bass2jax.partition_id_tensor())
        outs = bass2jax._bass_exec_p.bind(
            *operands,
            out_avals=tuple(out_avals),
            in_names=tuple(all_in_names),
            out_names=tuple(out_names),
            lowering_input_output_aliases=(),
            sim_require_finite=True,
            sim_require_nnan=True,
            nc=nc,
        )
        return tuple(outs)

    devices = jax.devices()[:N_CORES]
    mesh = Mesh(np.asarray(devices), ("core",))
    in_specs = (PartitionSpec("core"),) * (n_params + len(out_names))
    out_specs = (PartitionSpec("core"),) * len(out_names)
    sharded = jax.jit(
        shard_map(
            _body, mesh=mesh, in_specs=in_specs, out_specs=out_specs, check_rep=False
        ),
        keep_unused=True,
    )

    def run(in_maps):
        concat_in = [
            np.concatenate([m[name] for m in in_maps], axis=0) for name in in_names
        ]
        concat_zeros = [
            np.zeros((N_CORES * z.shape[0], *z.shape[1:]), z.dtype) for z in zero_outs
        ]
        out_arrs = sharded(*concat_in, *concat_zeros)
        return [
            {
                name: np.asarray(out_arrs[i]).reshape(
                    N_CORES, *out_avals[i].shape
                )[c]
                for i, name in enumerate(out_names)
            }
            for c in range(N_CORES)
        ]

    runner = {"run": run, "sharded": sharded, "in_names": in_names,
              "out_names": out_names, "out_avals": out_avals,
              "zero_outs": zero_outs, "mesh": mesh}
    _NC_CACHE["runner"] = runner
    return runner


def kernel(x, mask, wq, bq, wk, bk, wv, bv, wo, bo):
    global LAST_RESULT
    x = np.asarray(x, np.float32)
    wq = np.asarray(wq, np.float32)
    wk = np.asarray(wk, np.float32)
    wv = np.asarray(wv, np.float32)
    wo = np.asarray(wo, np.float32)
    bq = np.asarray(bq, np.float32)
    bk = np.asarray(bk, np.float32)
    bv = np.asarray(bv, np.float32)
    bo = np.asarray(bo, np.float32)

    in_maps = _make_in_maps(x, wq, bq, wk, bk, wv, bv, wo)
    results = _get_runner()["run"](in_maps)
    LAST_RESULT = results

    out = np.empty((B, S, D), np.float32)
    for b in range(B):
        out[b] = results[2 * b]["out"] + results[2 * b + 1]["out"] + bo
    return out


def _make_in_maps(x, wq, bq, wk, bk, wv, bv, wo):
    wdt = mybir.dt.np(CDT)
    in_maps = []
    for c in range(N_CORES):
        b, hg = c // 2, c % 2
        rows = slice(HD * hg, HD * hg + HD)
        in_maps.append(
            {
                "x": np.ascontiguousarray(x[b]),
                "wqt": np.ascontiguousarray(wq[rows, :].T).astype(wdt),
                "wkt": np.ascontiguousarray(wk[rows, :].T).astype(wdt),
                "wvt": np.ascontiguousarray(wv[rows, :].T).astype(wdt),
                "wot": np.ascontiguousarray(wo[:, rows].T).astype(wdt),
                "bq": np.ascontiguousarray(bq[rows].reshape(OTS, P).T),
                "bk": np.ascontiguousarray(bk[rows].reshape(OTS, P).T),
                "bv": np.ascontiguousarray(bv[rows].reshape(OTS, P).T),
            }
        )
    return in_maps


def bench(inputs, iters=20):
    """Time pure device execution with device-resident inputs.

    Returns (times_s, results_of_last_run). Wall-clocks each sharded call
    with block_until_ready; min over iters approximates HW exec time +
    fixed dispatch overhead.
    """
    import time as _time

    import jax
    from jax.sharding import NamedSharding, PartitionSpec

    x = np.asarray(inputs["x"], np.float32)
    wq = np.asarray(inputs["wq"], np.float32)
    wk = np.asarray(inputs["wk"], np.float32)
    wv = np.asarray(inputs["wv"], np.float32)
    wo = np.asarray(inputs["wo"], np.float32)
    bq = np.asarray(inputs["bq"], np.float32)
    bk = np.asarray(inputs["bk"], np.float32)
    bv = np.asarray(inputs["bv"], np.float32)
    in_maps = _make_in_maps(x, wq, bq, wk, bk, wv, bv, wo)

    r = _get_runner()
    sharded, in_names, zero_outs, mesh = (
        r["sharded"], r["in_names"], r["zero_outs"], r["mesh"],
    )
    sh = NamedSharding(mesh, PartitionSpec("core"))
    dev_args = [
        jax.device_put(
            np.concatenate([m[name] for m in in_maps], axis=0), sh
        )
        for name in in_names
    ] + [
        jax.device_put(
            np.zeros((N_CORES * z.shape[0], *z.shape[1:]), z.dtype), sh
        )
        for z in zero_outs
    ]
    # warmup (also triggers compile on first use)
    out = sharded(*dev_args)
    jax.block_until_ready(out)
    times = []
    for _ in range(iters):
        t0 = _time.perf_counter()
        out = sharded(*dev_args)
        jax.block_until_ready(out)
        times.append(_time.perf_counter() - t0)
    return times, out
